# revision 9
# baseline (speedup 1.0000x reference)
"""Trainium2 Bass kernel for NeuroVPR Vanilla SNN (3-layer LIF, T=3).

Data-parallel over batch: B=16384 -> 2048 per core x 8 cores.

Math (per timestep, per layer): v = (v_prev + h)/2; s = (v>=1); v *= (1-s).
The LIF recurrence is homogeneous and the decay is a power of 2, so each
layer runs in a scaled basis u_t = 2^t * 2c * v_t (c = weight prescale,
32 for L1 / 8 for L2-L3): u_t = m_{t-1} + 2^t*psum_t; s_t = (u_t >= th_t);
m_t = u_t*(1-s_t), th_t = 2^t * 2c.

Hidden spikes are stored as MASKS (mask = 1 - s, in {0,1} fp8) and W2/W3
are stored NEGATED: W.s = rowsum(W) - W.mask, with the rowsum folded into
each consumer's beta column (host-precomputed from the quantized
weights). This makes the m-update a 2x-rate VectorE tensor_tensor mult
(m = u*mask) instead of a slow scalar_tensor_tensor, and needs no
ScalarE Sign ops at all. L1's bias rides a ones row appended to x.

The m + hb add rides the TensorE for the late chains: a bf16 identity
matmul (stationary c*I128, host-provided) accumulates c*m into the same
PSUM group as the layer's GEMM, so t=2 needs NO ScalarE extract: spikes
compare PSUM directly against an immediate (L1) or a per-partition
threshold column (L2/L3, tensor_scalar with AP scalar; thcol =
16 - rowsum - SC2*b). All matmuls fp8e4 DoubleRow (216 ns at N=512)
except the identity accumulates (bf16, same issue rate).

Schedule: t=0/1 run as half-batch passes (2 double-bank psum groups, k
inner); t=2 runs as FOUR 512-column quarter-passes so the L2/L3(t2)
chains pipeline against the remaining L1 matmul stream and only the last
quarter's short chain trails the final matmul. L2/L3 matmul groups hook
into the k-loops of later passes so the in-order PE queue never waits on
their dependencies. The HAM throttle halves PE rate after ~68us of
activity, so the plan keeps added PE work minimal and leaves the
post-stream tail to Vector (unaffected by the throttle).
x half-tiles prefetch one pass ahead; the initial prefetch splits across
the sync and scalar HW DGE queues with w1's k=0 slab first on sync.
GpSimd issues no DMAs (its epilogue queue-drains are slow when used)
and does no compute (~8us per [128,512] op).
"""
import os
import numpy as np
import ml_dtypes

B, T, D = 16384, 3, 2752
DP = 2816          # D padded to 11*256
KD = DP // 256     # 11 DoubleRow contraction slabs
H, O = 256, 100
OP = 112           # O padded so the DoubleRow pair-stride is 16B-aligned
NCORES = 8
BC = B // NCORES   # 2048
NB = 512           # matmul free-dim block
WB = 1024          # half-pass LIF elementwise span (2 psum banks)
HB = BC // 2       # half-batch per L1 pass (1024)

SC1, SC2 = 32.0, 8.0    # weight prescale: L1; L2/L3
TH1, TH2 = 64.0, 32.0   # base thresholds (scaled x2 each timestep)

_compiled = None
last_results = None  # BassKernelResults of the most recent run (for profiling)


def _build():
    from contextlib import ExitStack
    import concourse.bass as bass
    import concourse.mybir as mybir
    import concourse.tile as tile
    from concourse import bacc

    f8 = mybir.dt.float8e4
    bf16 = mybir.dt.bfloat16
    f32 = mybir.dt.float32
    A = mybir.AluOpType
    DR = mybir.MatmulPerfMode.DoubleRow
    IDENT = mybir.ActivationFunctionType.Identity

    nc = bacc.Bacc("TRN2", target_bir_lowering=False, debug=False)
    x = nc.dram_tensor("x", [T, KD, 2, 128, HB * 2], f8, kind="ExternalInput").ap()
    w1 = nc.dram_tensor("w1", [128, KD * 2 * H], f8, kind="ExternalInput").ap()
    w2 = nc.dram_tensor("w2", [128, 2 * H], f8, kind="ExternalInput").ap()
    w3 = nc.dram_tensor("w3", [128, 2 * OP], f8, kind="ExternalInput").ap()
    bias = nc.dram_tensor("bias", [128, 16], f32, kind="ExternalInput").ap()
    ident = nc.dram_tensor("ident", [128, 384], bf16, kind="ExternalInput").ap()
    out = nc.dram_tensor("out", [O, BC], f32, kind="ExternalOutput").ap()

    with tile.TileContext(nc) as tc, ExitStack() as ctx:
        wp = ctx.enter_context(tc.tile_pool(name="wp", bufs=1))
        xp = ctx.enter_context(tc.tile_pool(name="xp", bufs=24))
        pp1 = ctx.enter_context(tc.tile_pool(name="pp1", bufs=3, space="PSUM"))
        pp23 = ctx.enter_context(tc.tile_pool(name="pp23", bufs=1, space="PSUM"))
        sp = ctx.enter_context(tc.tile_pool(name="sp", bufs=1))
        tp = ctx.enter_context(tc.tile_pool(name="tp", bufs=6))

        # ---- ACT warmup first: fully host-data-independent ----
        wu = wp.tile([128, 8], bf16)
        wub = wp.tile([128, 1], f32)
        nc.vector.memset(wu[:, :], 0.0)
        nc.vector.memset(wub[:, :], 0.0)
        nc.scalar.activation(wu[:, 0:4], wu[:, 4:8], IDENT, bias=wub[:, 0:1])

        # ---- weights / bias loads ----
        # w1 k=0 slab rides the sync HW queue FIRST (gates the first matmul);
        # the rest of w1 + small tensors ride the scalar HW queue ahead of
        # that queue's x tiles.  (Only sync/scalar have HW DGE queues.)
        w1t = wp.tile([128, KD * 2 * H], f8)
        nc.sync.dma_start(out=w1t[:, 0:512], in_=w1[:, 0:512])
        for c0, c1 in ((512, 2048), (2048, 4096), (4096, KD * 512)):
            nc.scalar.dma_start(out=w1t[:, c0:c1], in_=w1[:, c0:c1])
        w1o = w1t[:, :].rearrange("p (k two m) -> p k two m", k=KD, two=2)
        bt = wp.tile([128, 16], f32)
        nc.scalar.dma_start(out=bt[:, :], in_=bias[:, :])
        idt = wp.tile([128, 384], bf16)
        nc.scalar.dma_start(out=idt[:, :], in_=ident[:, :])
        I25 = idt[:, 0:128]     # 0.25 * I128
        I125 = idt[:, 128:256]  # 0.125 * I128
        I50 = idt[:, 256:384]   # 0.5 * I128
        w2t = wp.tile([128, 2 * H], f8)
        nc.scalar.dma_start(out=w2t[:, :], in_=w2[:, :])
        w2o = w2t[:, :].rearrange("p (two m) -> p two m", two=2)
        w3t = wp.tile([128, 2 * OP], f8)
        nc.scalar.dma_start(out=w3t[:, :], in_=w3[:, :])
        w3o = w3t[:, :].rearrange("p (two m) -> p two m", two=2)
        # bias column layout (host fills):
        # 0-5 beta1[t,h]; 6-9 beta2[t,h] t<2; 10-11 beta3[t] t<2;
        # 12-13 thcol2[h] = 16 - rs2 - SC2*b2; 14 thcol3 = 16 - rs3 - SC2*b3
        B1 = lambda t, h: bt[:, 2 * t + h: 2 * t + h + 1]
        B2 = lambda t, h: bt[:, 6 + 2 * t + h: 6 + 2 * t + h + 1]
        B3 = lambda t: bt[:, 10 + t: 11 + t]
        TC2 = lambda h: bt[:, 12 + h: 13 + h]
        TC3 = bt[:, 14:15]

        # ---- persistent state (m = scaled membrane, written at t=0) ----
        m1 = [sp.tile([128, BC], bf16, tag=f"m1_{h}", name=f"m1_{h}")
              for h in range(2)]
        m2 = [sp.tile([128, BC], bf16, tag=f"m2_{h}", name=f"m2_{h}")
              for h in range(2)]
        m3 = sp.tile([128, BC], bf16, tag="m3")
        s1 = sp.tile([128, 2 * BC], f8, tag="s1")
        s2 = sp.tile([128, 2 * BC], f8, tag="s2")
        s1r = s1[:, :].rearrange("p (two n) -> p two n", two=2)
        s2r = s2[:, :].rearrange("p (two n) -> p two n", two=2)
        outsb = sp.tile([128, BC], f32, tag="outsb")

        xt = {}  # (t, k, half) -> x tile handle [128, 2*HB]

        def x_fetch(t, k, half, q=None):
            xt[t, k, half] = xp.tile([128, 2 * HB], f8, tag="x",
                                     name=f"x_{t}_{k}_{half}")
            (q or nc.sync).dma_start(out=xt[t, k, half][:, :],
                                     in_=x[t, k, half, :, :])

        # ext scale for mask-basis L2/L3 psums at t<2: u = 2^(t+1)*ps + beta
        SSC = [2.0, 4.0]

        def lif(ps, m_ap, s_ap, bcol, th, t, P=128, sc=None, m_in_psum=False):
            """Mask-basis scaled-LIF on one [P, WB] psum span (t < T-1):
            ScalarE extract (+m via tensor_tensor unless the identity
            matmul already accumulated it into PSUM), then VectorE
            mask = (u < th) and m = u * mask."""
            hb = tp.tile([128, WB], bf16, tag="hb", name="hb")[:P, :]
            nc.scalar.activation(hb, ps, IDENT, bias=bcol[:P, :],
                                 scale=float(2 ** t) if sc is None else sc)
            if t == 0 or m_in_psum:
                u = hb
            else:
                u = tp.tile([128, WB], bf16, tag="u", name="u")[:P, :]
                nc.vector.tensor_tensor(u, m_ap, hb, A.add)
            if s_ap is not None:
                nc.vector.tensor_scalar(s_ap, u, th * 2 ** t, None, A.is_lt)
                nc.vector.tensor_tensor(m_ap, u, s_ap, A.mult)
            else:
                nc.vector.scalar_tensor_tensor(m_ap, u, th * 2 ** t, u,
                                               A.is_lt, A.mult)

        def l1_pass(t, half, hooks=None, m_imm=False):
            """One half-batch L1 pass (t < T-1): 2 double-bank psum groups,
            k inner. Prefetches the next pass's x tiles; `hooks[k]` emits
            interleaved L2/L3 work. With m_imm, 0.5*m1 is accumulated into
            PSUM by an identity matmul (ext scale 2 absorbs it at t=1)."""
            boff = half * HB
            ps = [pp1.tile([128, WB], f32, tag="ps1", name=f"ps1_{t}_{half}_{h}")
                  for h in range(2)]
            for k in range(KD):
                for fn in (hooks or {}).get(k, []):
                    fn()
                xr = xt[t, k, half][:, :].rearrange("p (two n) -> p two n", two=2)
                for h in range(2):
                    for b in range(2):
                        nc.tensor.matmul(
                            ps[h][:, b * NB:(b + 1) * NB],
                            w1o[:, k, :, h * 128:(h + 1) * 128],
                            xr[:, :, b * NB:(b + 1) * NB],
                            start=(k == 0),
                            stop=(k == KD - 1) and not m_imm, perf_mode=DR,
                            skip_group_check=True)
                if half == 0:
                    x_fetch(t, k, 1)
                else:
                    x_fetch(t + 1, k, 0)
            if m_imm:
                for h in range(2):
                    for b in range(2):
                        bs = slice(boff + b * NB, boff + (b + 1) * NB)
                        nc.tensor.matmul(ps[h][:, b * NB:(b + 1) * NB],
                                         I50, m1[h][:, bs], start=False,
                                         stop=True, skip_group_check=True)
            for h in range(2):
                bs = slice(boff, boff + WB)
                lif(ps[h][:, :], m1[h][:, bs],
                    s1[:, h * BC + boff: h * BC + boff + WB],
                    B1(t, h), TH1, t, m_in_psum=m_imm)

        def l1_quarter(q, hooks=None):
            """One 512-column L1 quarter at t=T-1: k inner, 2 MMs per slab
            into the two banks of one [128,1024] psum tile, plus a 0.25*I
            matmul accumulating m1. The spike mask comes straight from
            PSUM (u/4 vs TH1); no m1 update needed."""
            half, b = q // 2, q % 2
            qs = slice(q * NB, (q + 1) * NB)
            ps = pp1.tile([128, WB], f32, tag="ps1", name=f"psq_{q}")
            for k in range(KD):
                for fn in (hooks or {}).get(k, []):
                    fn()
                xr = xt[T - 1, k, half][:, :].rearrange(
                    "p (two n) -> p two n", two=2)
                for h in range(2):
                    nc.tensor.matmul(
                        ps[:, h * NB:(h + 1) * NB],
                        w1o[:, k, :, h * 128:(h + 1) * 128],
                        xr[:, :, b * NB:(b + 1) * NB],
                        start=(k == 0), stop=False, perf_mode=DR,
                        skip_group_check=True)
            for h in range(2):
                nc.tensor.matmul(ps[:, h * NB:(h + 1) * NB], I25,
                                 m1[h][:, qs], start=False, stop=True,
                                 skip_group_check=True)
            for h in range(2):
                nc.vector.tensor_scalar(
                    s1[:, h * BC + q * NB: h * BC + (q + 1) * NB],
                    ps[:, h * NB:(h + 1) * NB], TH1, None, A.is_lt)

        def l2_one(t, h, bp):
            """Full-span L2 chain (t < T-1): negated-W GEMM + 0.5*m2
            identity accumulate at t=1; psum from the shared ps23 ring."""
            bs = slice(bp * WB, (bp + 1) * WB)
            ps2 = pp23.tile([128, WB], f32, tag="ps23", name=f"ps2_{t}_{h}_{bp}")
            for b in range(2):
                nc.tensor.matmul(
                    ps2[:, b * NB:(b + 1) * NB],
                    w2o[:, :, h * 128:(h + 1) * 128],
                    s1r[:, :, (2 * bp + b) * NB:(2 * bp + b + 1) * NB],
                    start=True, stop=(t == 0), perf_mode=DR,
                    skip_group_check=True)
                if t == 1:
                    # ext scale is 4 at t=1, so 0.25*I keeps m2 coefficient 1
                    nc.tensor.matmul(
                        ps2[:, b * NB:(b + 1) * NB], I25,
                        m2[h][:, bp * WB + b * NB: bp * WB + (b + 1) * NB],
                        start=False, stop=True, skip_group_check=True)
            lif(ps2[:, :], m2[h][:, bs],
                s2[:, h * BC + bp * WB: h * BC + (bp + 1) * WB],
                B2(t, h), TH2, t, sc=SSC[t], m_in_psum=(t == 1))

        def l3_one(t, bp):
            """Full-span L3 chain (t < T-1): updates m3 only."""
            bs = slice(bp * WB, (bp + 1) * WB)
            ps3 = pp23.tile([128, WB], f32, tag="ps23", name=f"ps3_{t}_{bp}")
            for b in range(2):
                nc.tensor.matmul(ps3[:OP, b * NB:(b + 1) * NB], w3o[:, :, :],
                                 s2r[:, :, (2 * bp + b) * NB:(2 * bp + b + 1) * NB],
                                 start=True, stop=(t == 0), perf_mode=DR,
                                 skip_group_check=True)
                if t == 1:
                    # K=112 contraction avoids m3's uninitialized rows
                    nc.tensor.matmul(
                        ps3[:OP, b * NB:(b + 1) * NB], I25[:OP, :OP],
                        m3[:OP, bp * WB + b * NB: bp * WB + (b + 1) * NB],
                        start=False, stop=True, skip_group_check=True)
            lif(ps3[:OP, :], m3[:OP, bs], None, B3(t), TH2, t,
                P=OP, sc=SSC[t], m_in_psum=(t == 1))

        # ---- t=2 quarter chains (extract-free: thresholds from PSUM) ----
        cht = {}  # q -> shared L2/L3 chain psum tile

        def c_l2(q):
            qs = slice(q * NB, (q + 1) * NB)
            ch = pp23.tile([128, WB], f32, tag="ps23", name=f"chq_{q}")
            cht[q] = ch
            for h in range(2):
                nc.tensor.matmul(ch[:, h * NB:(h + 1) * NB],
                                 w2o[:, :, h * 128:(h + 1) * 128],
                                 s1r[:, :, q * NB:(q + 1) * NB],
                                 start=True, stop=False, perf_mode=DR,
                                 skip_group_check=True)
                nc.tensor.matmul(ch[:, h * NB:(h + 1) * NB], I125,
                                 m2[h][:, qs], start=False, stop=True,
                                 skip_group_check=True)
            for h in range(2):
                nc.vector.tensor_scalar(
                    s2[:, h * BC + q * NB: h * BC + (q + 1) * NB],
                    ch[:, h * NB:(h + 1) * NB], TC2(h), None, A.is_lt)

        def c_l3(q):
            qs = slice(q * NB, (q + 1) * NB)
            ch = cht[q]
            nc.tensor.matmul(ch[:OP, 0:NB], w3o[:, :, :],
                             s2r[:, :, q * NB:(q + 1) * NB],
                             start=True, stop=False, perf_mode=DR,
                             skip_group_check=True)
            nc.tensor.matmul(ch[:OP, 0:NB], I125[:OP, :OP], m3[:OP, qs],
                             start=False, stop=True, skip_group_check=True)
            nc.vector.tensor_scalar(outsb[:OP, qs], ch[:OP, 0:NB],
                                    TC3[:OP, :], None, A.is_ge)
            (nc.sync if q % 2 == 0 else nc.scalar).dma_start(
                out=out[:, qs], in_=outsb[:O, qs])

        # ---- initial prefetch: sync gets w1 k0 + most x; scalar takes
        # w1's tail slabs + small tensors then two late x tiles ----
        for k in (0, 1, 2, 3, 4, 5, 6, 8, 10):
            x_fetch(0, k, 0)
        x_fetch(0, 7, 0, nc.scalar)
        x_fetch(0, 9, 0, nc.scalar)

        l1_pass(0, 0)
        l1_pass(0, 1)
        l1_pass(1, 0, hooks={3: [lambda: l2_one(0, 0, 0)],
                             7: [lambda: l2_one(0, 1, 0)]})
        l1_pass(1, 1, hooks={1: [lambda: l2_one(0, 0, 1)],
                             4: [lambda: l2_one(0, 1, 1)],
                             7: [lambda: l3_one(0, 0)],
                             10: [lambda: l2_one(1, 0, 0)]},
                m_imm=True)
        # t=2: four quarter passes; half1 x tiles fetched across q0/q1
        q0h = {1: [lambda: l3_one(0, 1)],
               4: [lambda: l2_one(1, 1, 0)],
               8: [lambda: l2_one(1, 0, 1)]}
        for k in range(0, KD, 2):
            q0h.setdefault(k, []).append(
                lambda j=k // 2: x_fetch(2, j, 1))
        q1h = {2: [lambda: l3_one(1, 0)],
               5: [lambda: l2_one(1, 1, 1)],
               9: [lambda: c_l2(0)]}
        for k in (0, 2, 4, 6, 8):
            q1h.setdefault(k, []).append(lambda j=6 + k // 2: x_fetch(2, j, 1))
        l1_quarter(0, hooks=q0h)
        l1_quarter(1, hooks=q1h)
        l1_quarter(2, hooks={2: [lambda: c_l3(0)],
                             5: [lambda: l3_one(1, 1)],
                             9: [lambda: c_l2(1)]})
        l1_quarter(3, hooks={2: [lambda: c_l3(1)],
                             6: [lambda: c_l2(2)]})
        c_l3(2)
        c_l2(3)
        c_l3(3)

    nc.compile()
    return nc


def kernel(dvs, W1, b1, W2, b2, W3, b3):
    global _compiled, last_results
    from concourse.bass_utils import run_bass_kernel_spmd

    if _compiled is None:
        _compiled = _build()
    nc = _compiled

    f8 = ml_dtypes.float8_e4m3

    def q8(a, scale):
        return np.clip(a * scale, -240.0, 240.0).astype(f8)

    # x: [B, T, D] -> fp8 [T, KD, 128, 2, B]  (d = k*256 + two*128 + p)
    x8 = q8(dvs, 1.0).transpose(1, 2, 0)          # [T, D, B]
    X = np.zeros((T, KD, 2, 128, B), dtype=f8)
    X.reshape(T, DP, B)[:, :D, :] = x8
    X.reshape(T, DP, B)[:, D, :] = f8(1.0)        # bias row (w1 row D = c1*b1)
    X = np.ascontiguousarray(X.transpose(0, 1, 3, 2, 4))  # [T, KD, 128, 2, B]

    # w1: [DP, H] scaled by SC1 -> [128, KD, 2, H]
    w1p = np.zeros((KD, 2, 128, H), dtype=f8)
    w1p.reshape(DP, H)[:D, :] = q8(W1.T, SC1)
    w1p.reshape(DP, H)[D, :] = q8(b1, SC1)
    w1p = np.ascontiguousarray(w1p.transpose(2, 0, 1, 3)).reshape(128, KD * 2 * H)
    # w2/w3 scaled by SC2, quantized, then NEGATED (mask-basis consumption)
    w2q = q8(W2.T, SC2)                            # [H, H] j-major
    w2p = np.ascontiguousarray(
        (-w2q).reshape(2, 128, H).transpose(1, 0, 2)).reshape(128, 2 * H)
    w3q = np.zeros((H, OP), dtype=f8)
    w3q[:, :O] = q8(W3.T, SC2)
    w3p = np.ascontiguousarray(
        (-w3q).reshape(2, 128, OP).transpose(1, 0, 2)).reshape(128, 2 * OP)

    # bias/threshold columns; row-sum corrections use the quantized weights.
    # W.s01 = rowsum(W) - W.mask => hb = 2^t*(2*psum_neg + 2*(rs + SC2*b))
    rs2 = w2q.astype(np.float64).sum(axis=0)       # [H]
    rs3 = w3q.astype(np.float64).sum(axis=0)       # [OP]
    bc = np.zeros((128, 16), dtype=np.float32)
    for t in range(2):
        p2 = float(2 ** t)
        for h in range(2):
            bc[:, 6 + 2 * t + h] = 2 * p2 * (rs2[h * 128:(h + 1) * 128]
                                             + SC2 * b2[h * 128:(h + 1) * 128])
        bc[:OP, 10 + t] = 2 * p2 * rs3
        bc[:O, 10 + t] += 2 * p2 * SC2 * b3
    # t=2 thresholds straight from PSUM (psum = -W.mask + 0.125*m):
    # spike <=> 8*psum + beta2(t2) >= 128 <=> psum >= 16 - rs - SC2*b
    for h in range(2):
        bc[:, 12 + h] = 16.0 - rs2[h * 128:(h + 1) * 128] \
            - SC2 * b2[h * 128:(h + 1) * 128]
    bc[:OP, 14] = 16.0 - rs3
    bc[:O, 14] -= SC2 * b3

    idm = np.zeros((128, 384), dtype=ml_dtypes.bfloat16)
    ii = np.arange(128)
    idm[ii, ii] = 0.25
    idm[ii, 128 + ii] = 0.125
    idm[ii, 256 + ii] = 0.5

    in_maps = []
    for c in range(NCORES):
        xc = X[:, :, :, :, c * BC:(c + 1) * BC]    # [T, KD, 128, 2, BC]
        xc = np.ascontiguousarray(
            xc.reshape(T, KD, 128, 2, 2, HB).transpose(0, 1, 4, 2, 3, 5)
        ).reshape(T, KD, 2, 128, 2 * HB)           # [T, KD, half, 128, 2*HB]
        in_maps.append({"x": xc, "w1": w1p, "w2": w2p, "w3": w3p, "bias": bc,
                        "ident": idm})

    trace = bool(os.environ.get("SNN_TRACE"))
    last_results = run_bass_kernel_spmd(nc, in_maps, core_ids=list(range(NCORES)),
                                        trace=trace)
    outp = np.empty((B, O), dtype=np.float32)
    for c in range(NCORES):
        outp[c * BC:(c + 1) * BC, :] = last_results.results[c]["out"].T
    return outp
